# revision 30
# baseline (speedup 1.0000x reference)
"""AUGRU (attention-gated GRU) layer on 8 Trainium2 NeuronCores.

Strategy
--------
Data-parallel: batch 2048 is sharded 256/core across 8 cores; the six small
[128,128] weight matrices are replicated; the T=200 recurrence runs locally
per core (no collectives).

On-chip layout is fully "transposed": hidden state and all per-step tensors
live as [U=128 partitions, B_local=256 free], so every matmul is
`out[U,B] = W.T @ rhs[U,B]` with the weight matrix (as given) as the
stationary operand.  The host pre-transposes interest_states to
[T, U, B_local] per core so time-step slices DMA directly into matmul-rhs
layout, and pre-broadcasts attention scores across partitions.

Everything on-chip is fp16 (inputs, weights, state) with fp32 PSUM
accumulation — fp16 matmuls run at full PE rate and the 10-bit mantissa
keeps the recurrence error ~1e-3.  All inputs here are bounded (x~N(0,1),
a in (0,1), gates/h in [-1,1]) so fp16 range is safe.

Per step t:
    zu|zr = W_xu.T@x + W_hu.T@q + W_hu_neg.T@s   (PSUM accumulate; same for r)
    zc    = W_xc.T@x + W_hc.T@(r*h)
    u,r   = sigmoid(z + b)     (ACT reads PSUM directly, fused bias)
    c     = tanh(zc + b)
    au    = a * u              (a pre-broadcast from HBM)
    s     = (au - 1) * h       (fused scalar_tensor_tensor; s == -(1-au)*h)
    q     = au * c
    h'    = q - s              (materialized off the critical path)
Next step's h-matmuls consume q and s separately (W_h*_neg are host-negated
copies), keeping the h-materialization off the serial recurrence chain.

split=2: the serial chain (tanh -> q -> matmul -> sigmoid_r -> r*h -> matmul)
is the wall-clock floor, so batch columns are split into two independent
half-recurrences (b 0:128 / 128:256) whose chains pipeline against each other
on the engines.  Each half owns its own PSUM banks so PSUM bank-collision
serialization cannot couple the halves.
"""

import os
from contextlib import ExitStack

import numpy as np

from concourse import bacc
import concourse.mybir as mybir
import concourse.tile as tile
from concourse.bass_utils import run_bass_kernel_spmd

B, T_FULL, U = 2048, 200, 128
NCORES = 8
BL = B // NCORES  # 256

F32 = mybir.dt.float32
F16 = mybir.dt.float16
AF = mybir.ActivationFunctionType
OP = mybir.AluOpType

# Last run's results (for test harness introspection)
LAST_RESULTS = None


def _build_nc(T: int, tc_steps: int, repeat: int = 1, split: int = 2,
              fused_ur: bool = True, au_eng: str = "dve", hsub_eng: str = "dve",
              s_eng: str = "dve", chain_prio: int | None = 0, pair_u: bool = False,
              gates_bufs: int = 2, state_bufs: int = 3, x_bufs: int = 2):
    nc = bacc.Bacc("TRN2", target_bir_lowering=False, debug=False)

    assert BL % split == 0
    HB = BL // split

    xT = nc.declare_dram_parameter("xT", [T, U, BL], F16, isOutput=False)
    aB = nc.declare_dram_parameter("aB", [T, U, BL], F16, isOutput=False)
    w_all = nc.declare_dram_parameter("w_all", [U, 8 * U], F16, isOutput=False)
    bias = nc.declare_dram_parameter("bias", [U, 3], F32, isOutput=False)
    h_out = nc.declare_dram_parameter("h_out", [U, BL], F32, isOutput=True)

    n_chunks = T // tc_steps
    assert n_chunks * tc_steps == T

    with ExitStack() as ctx:
        tc = ctx.enter_context(tile.TileContext(nc))
        consts = ctx.enter_context(tc.tile_pool(name="consts", bufs=1))
        xpool = ctx.enter_context(tc.tile_pool(name="x", bufs=x_bufs))
        apool = ctx.enter_context(tc.tile_pool(name="a", bufs=x_bufs))
        gates = ctx.enter_context(tc.tile_pool(name="gates", bufs=gates_bufs))
        state = ctx.enter_context(tc.tile_pool(name="state", bufs=state_bufs))
        if pair_u:
            urp_pool = ctx.enter_context(
                tc.tile_pool(name="ps_urp", bufs=2, space="PSUM"))
        else:
            ur_pools = [
                ctx.enter_context(
                    tc.tile_pool(name=f"ps_ur{hb}", bufs=2, space="PSUM"))
                for hb in range(split)
            ]
        c_pools = [
            ctx.enter_context(tc.tile_pool(name=f"ps_c{hb}", bufs=2, space="PSUM"))
            for hb in range(split)
        ]

        w_sb = consts.tile([U, 8 * U], F16, tag="w")
        nc.sync.dma_start(w_sb[:], w_all[:])
        bias_sb = consts.tile([U, 3], F32, tag="bias")
        nc.sync.dma_start(bias_sb[:], bias[:])
        sw = {}
        for i, name in enumerate(["xu", "xr", "xc", "hu", "hr", "hc", "hun", "hrn"]):
            sw[name] = w_sb[:, i * U : (i + 1) * U]
        sb_bu = bias_sb[:, 0:1]
        sb_br = bias_sb[:, 1:2]
        sb_bc = bias_sb[:, 2:3]

        xT_ap = xT[:]
        aB_ap = aB[:]

        q_prev = None   # fp16 [U, BL] (written per half)
        s_prev = None   # fp16 [U, BL]; None while h == q_prev
        h_cur = None

        for ic in range(n_chunks * repeat):
            ic = ic % n_chunks
            t0 = ic * tc_steps
            x_ch = xpool.tile([U, tc_steps, BL], F16, tag="x_ch")
            nc.sync.dma_start(
                x_ch[:], xT_ap[t0 : t0 + tc_steps, :, :].rearrange("t u b -> u t b")
            )
            a_ch = apool.tile([U, tc_steps, BL], F16, tag="a_ch")
            nc.sync.dma_start(
                a_ch[:], aB_ap[t0 : t0 + tc_steps, :, :].rearrange("t u b -> u t b")
            )

            for tl in range(tc_steps):
                t = t0 + tl
                if repeat > 1:
                    t = 0 if q_prev is None else (1 if s_prev is None else 2)
                x_t = x_ch[:, tl, :]
                a_t = a_ch[:, tl, :]

                # materialize h for this step (off the critical chain)
                if t == 0:
                    h_cur = None
                elif s_prev is None:
                    h_cur = q_prev
                else:
                    h_cur = [state.tile([U, HB], F16, tag=f"h{hb}", name=f"h{hb}")
                             for hb in range(split)]
                    heng = nc.gpsimd if hsub_eng == "gp" else nc.vector
                    for hb in range(split):
                        heng.tensor_sub(h_cur[hb][:], q_prev[hb][:],
                                        s_prev[hb][:])

                u_t = [gates.tile([U, HB], F16, tag=f"u{hb}", name=f"u{hb}")
                       for hb in range(split)]
                r_t = [gates.tile([U, HB], F16, tag=f"r{hb}", name=f"r{hb}")
                       for hb in range(split)] if t > 0 else None
                c_t = [gates.tile([U, HB], F16, tag=f"c{hb}", name=f"c{hb}")
                       for hb in range(split)]
                rh_t = [gates.tile([U, HB], F16, tag=f"rh{hb}", name=f"rh{hb}")
                        for hb in range(split)] if t > 0 else None
                au_t = [gates.tile([U, HB], F16, tag=f"au{hb}", name=f"au{hb}")
                        for hb in range(split)]
                q_t = [state.tile([U, HB], F16, tag=f"q{hb}", name=f"q{hb}")
                       for hb in range(split)]
                s_t = [state.tile([U, HB], F16, tag=f"s{hb}", name=f"s{hb}")
                       for hb in range(split)] if t > 0 else None

                if pair_u:
                    # one 2-bank tile; each half's [zu|zr] in its own bank
                    # (bank 2KB = 512 f32, half uses 256, rest padding) so
                    # PE-write/ACT-read pairs never serialize across halves,
                    # while sigma_u can read both zu regions in one strided op
                    purp = urp_pool.tile([U, split, 512], F32, tag="p_urp",
                                         name="p_urp")
                    u_full = gates.tile([U, split, HB], F16, tag="u",
                                        name="u_full")
                for hb in range(split):
                    sl = slice(hb * HB, (hb + 1) * HB)
                    if pair_u:
                        pur = purp[:, hb, 0 : 2 * HB]
                    else:
                        pur = ur_pools[hb].tile([U, 2 * HB], F32, tag=f"p_ur{hb}", name=f"p_ur{hb}")
                    pc = c_pools[hb].tile([U, HB], F32, tag=f"p_c{hb}", name=f"p_c{hb}")

                    # ---- gate pre-activations (PSUM f32 accumulate) ----
                    # x-side first (ready early), then s-side, then the
                    # q-side last: only the q-matmuls sit on the serial
                    # recurrence chain.
                    nc.tensor.matmul(pur[:, 0:HB], sw["xu"], x_t[:, sl],
                                     start=True, stop=False)
                    nc.tensor.matmul(pur[:, HB:], sw["xr"], x_t[:, sl],
                                     start=False, stop=(t == 0))
                    nc.tensor.matmul(pc[:], sw["xc"], x_t[:, sl],
                                     start=True, stop=(t == 0))
                    if t > 0:
                        if s_prev is not None:
                            nc.tensor.matmul(pur[:, 0:HB], sw["hun"],
                                             s_prev[hb][:], start=False, stop=False)
                            nc.tensor.matmul(pur[:, HB:], sw["hrn"],
                                             s_prev[hb][:], start=False, stop=False)
                        # hr@q first: sigma_r (serial chain) fires 1 matmul
                        # earlier; hu@q only feeds the off-chain sigma_u
                        nc.tensor.matmul(pur[:, HB:], sw["hr"], q_prev[hb][:],
                                         start=False, stop=False)
                        nc.tensor.matmul(pur[:, 0:HB], sw["hu"], q_prev[hb][:],
                                         start=False, stop=True)

                    # ---- activations / elementwise, this half ----
                    # chain-critical ops get elevated scheduler priority so
                    # off-chain work never queues ahead of them on an engine
                    r_hb = None
                    if t > 0:
                        with tc.high_priority(chain_prio):
                            nc.scalar.activation(r_t[hb][:], pur[:, HB:],
                                                 AF.Sigmoid, bias=sb_br[:])
                        r_hb = r_t[hb][:]
                    if not pair_u:
                        nc.scalar.activation(u_t[hb][:], pur[:, 0:HB],
                                             AF.Sigmoid, bias=sb_bu[:])

                    if t > 0:
                        hc_hb = h_cur[hb][:]
                        with tc.high_priority(chain_prio):
                            nc.vector.tensor_mul(rh_t[hb][:], r_hb, hc_hb)
                            nc.tensor.matmul(pc[:], sw["hc"], rh_t[hb][:],
                                             start=False, stop=True)

                    if pair_u and hb == 0:
                        # single sigmoid over both halves' zu regions
                        nc.scalar.activation(u_full[:], purp[:, :, 0:HB],
                                             AF.Sigmoid, bias=sb_bu[:])
                    u_hb = u_full[:, hb, :] if pair_u else u_t[hb][:]
                    aeng = nc.gpsimd if au_eng == "gp" else nc.vector
                    aeng.tensor_mul(au_t[hb][:], a_t[:, sl], u_hb)

                    with tc.high_priority(chain_prio):
                        nc.scalar.activation(c_t[hb][:], pc[:], AF.Tanh,
                                             bias=sb_bc[:])
                        nc.vector.tensor_mul(q_t[hb][:], au_t[hb][:], c_t[hb][:])

                    if t > 0:
                        if s_eng == "dve":
                            nc.vector.scalar_tensor_tensor(
                                s_t[hb][:], au_t[hb][:], 1.0, hc_hb,
                                OP.subtract, OP.mult
                            )
                        else:
                            # GPSIMD lacks scalar_tensor_tensor:
                            # s = (au-1)*h == au*h - h in two TT ops
                            tmp_t = gates.tile([U, HB], F16, tag=f"sx{hb}",
                                               name=f"sx{hb}")
                            nc.gpsimd.tensor_mul(tmp_t[:], au_t[hb][:], hc_hb)
                            nc.gpsimd.tensor_sub(s_t[hb][:], tmp_t[:], hc_hb)

                q_prev, s_prev = q_t, s_t

        # ---- epilogue: h_T = q - s, in f32 ----
        h_fin = state.tile([U, BL], F32, tag="h_fin")
        for hb in range(split):
            sl = slice(hb * HB, (hb + 1) * HB)
            if s_prev is None:
                nc.vector.tensor_copy(h_fin[:, sl], q_prev[hb][:])
            else:
                nc.vector.tensor_sub(h_fin[:, sl], q_prev[hb][:], s_prev[hb][:])
        nc.sync.dma_start(h_out[:], h_fin[:])

    nc.compile()
    return nc


def _prepare_in_maps(interest_states, attention_scores, W_xu, b_xu, W_hu, b_hu,
                     W_xr, b_xr, W_hr, b_hr, W_xc, b_xc, W_hc, b_hc, T: int):
    f32 = np.float32
    f16 = np.float16

    w_hu = np.asarray(W_hu, f32).astype(f16)
    w_hr = np.asarray(W_hr, f32).astype(f16)
    w_cat = np.concatenate([
        np.asarray(W_xu, f32).astype(f16),
        np.asarray(W_xr, f32).astype(f16),
        np.asarray(W_xc, f32).astype(f16),
        w_hu, w_hr,
        np.asarray(W_hc, f32).astype(f16),
        -w_hu, -w_hr,
    ], axis=1)  # [U, 8U]
    bu = np.asarray(b_xu, f32) + np.asarray(b_hu, f32)
    br = np.asarray(b_xr, f32) + np.asarray(b_hr, f32)
    bc = np.asarray(b_xc, f32) + np.asarray(b_hc, f32)
    bias_cat = np.stack([bu, br, bc], axis=1)  # [U, 3]
    fused_ur = bool(np.array_equal(bu, br))

    xs = np.asarray(interest_states, f32)
    at = np.asarray(attention_scores, f32)

    in_maps = []
    for c in range(NCORES):
        lo, hi = c * BL, (c + 1) * BL
        xTc = np.ascontiguousarray(
            xs[lo:hi, :T, :].transpose(1, 2, 0).astype(f16))  # [T, U, BL]
        aT = np.ascontiguousarray(at[lo:hi, :T, 0].T).astype(f16)  # [T, BL]
        aBc = np.ascontiguousarray(np.broadcast_to(aT[:, None, :], (T, U, BL)))
        in_maps.append({
            "xT": xTc, "aB": aBc, "w_all": w_cat, "bias": bias_cat,
        })
    return in_maps, fused_ur


def kernel(interest_states, attention_scores, W_xu, b_xu, W_hu, b_hu,
           W_xr, b_xr, W_hr, b_hr, W_xc, b_xc, W_hc, b_hc):
    global LAST_RESULTS
    T = int(os.environ.get("AUGRU_T", T_FULL))
    tc_steps = int(os.environ.get("AUGRU_TC", 10))
    split = int(os.environ.get("AUGRU_SPLIT", 2))
    trace = os.environ.get("AUGRU_TRACE", "0") == "1"
    tmpdir = os.environ.get("AUGRU_TMPDIR") or None

    in_maps, _ = _prepare_in_maps(
        interest_states, attention_scores, W_xu, b_xu, W_hu, b_hu,
        W_xr, b_xr, W_hr, b_hr, W_xc, b_xc, W_hc, b_hc, T)
    nc = _build_nc(T, tc_steps, split=split, fused_ur=False)

    res = run_bass_kernel_spmd(
        nc, in_maps, core_ids=list(range(NCORES)), trace=trace, tmpdir=tmpdir
    )
    LAST_RESULTS = res

    out = np.empty((B, U), np.float32)
    for c in range(NCORES):
        out[c * BL : (c + 1) * BL] = res.results[c]["h_out"].T
    return out
